# revision 1
# baseline (speedup 1.0000x reference)
"""Trainium2 Bass kernel for DisentangleStaticNoiseLoss (NT-Xent style loss).

Math (matches the jax reference):
    x   : [K=8192, D=128] stacked embeddings (N=8 blocks of BS=1024)
    z   : row-normalized x;  S = (z @ z.T) / 0.5
    row i (block b, sample r): positives = S[i, r + b'*BS] for b' != b,
    negatives = all j with j % BS != r.
    loss = mean over (i, pos) of [log(exp(pos) + sum_neg exp(neg)) - pos]

Sharding: 8 cores, each handles 1024 rows of S. Each core receives the FULL
x rotated so its own 1024 rows come first (host-side np.roll), which makes
the SPMD program identical on every core:
  - local column block B=0 holds the self-similarity diagonal,
  - positives sit on the diagonals of local column blocks B=1..7 at
    compile-time-known offsets.
Since cosine/0.5 is bounded by [-2, 2], exp never overflows and no max
subtraction is needed; A_i = F_i - P_i (full row-sum minus the 8 diagonal
terms) is exactly the negatives' exp-sum.

Engine budget per core (the exp of all K^2/8 similarities dominates):
  ACT: 32x Exp[128,2048] from PSUM + 2x Ln/Exp (rsqrt) + 2x Ln (finale)
  PE : 128 bf16 matmuls [128x128x512] (z in bf16; loss tolerates ~1e-5)
  DVE: 64 squared-norm accums, 8 broadcast scales, 64 diagonal extractions
  POOL: 32 row-sum accums of exp(S) supertiles
  DMA: input load, bf16 z bounce to DRAM, DMA-transpose back to [D, K]

Each core emits one fp32 partial sum; the host adds them and divides by
K*(N-1).
"""

import sys

import numpy as np

if "/opt/trn_rl_repo" not in sys.path:
    sys.path.insert(0, "/opt/trn_rl_repo")

N = 8
BS = 1024
D = 128
K = N * BS          # 8192
NCORES = 8
ROWS = K // NCORES  # 1024 rows per core
MT = ROWS // 128    # 8 m-tiles of 128 rows
SUP = 2048          # PSUM supertile columns (4 banks)
NSUP = K // SUP     # 4 supertiles per m-tile
TEMP_SCALE = 2.0    # 1 / temperature

_NC_CACHE = {}


def _build_nc():
    import concourse.bacc as bacc
    import concourse.bass as bass
    import concourse.tile as tile
    from concourse import mybir

    f32 = mybir.dt.float32
    bf16 = mybir.dt.bfloat16
    AX = mybir.AxisListType
    OP = mybir.AluOpType
    AF = mybir.ActivationFunctionType

    nc = bacc.Bacc("TRN2", target_bir_lowering=False, debug=False)
    xf = nc.declare_dram_parameter("xf", [K, D], f32, isOutput=False)
    ident = nc.declare_dram_parameter("ident", [128, 128], f32, isOutput=False)
    outp = nc.declare_dram_parameter("loss_out", [1, 1], f32, isOutput=True)

    with tile.TileContext(nc) as tc:
        with (
            tc.tile_pool(name="persist", bufs=1) as P,
            tc.tile_pool(name="work", bufs=3) as W,
            tc.tile_pool(name="dram", bufs=1, space="DRAM") as DP,
        ):
            zT = P.tile([128, K], bf16, tag="zT")       # z transposed: [D, K]
            idsb = P.tile([128, 128], f32, tag="idsb")  # identity matrix
            gbuf = P.tile([128, 8 * MT], f32, tag="gbuf")       # exp(pos logit)
            fcols = P.tile([128, MT * NSUP], f32, tag="fcols")  # partial row sums
            n2all = P.tile([128, 64], f32, tag="n2all")         # row norms^2
            scall = P.tile([128, 64], f32, tag="scall")         # 1/row norms
            zdram = DP.tile([K, D], bf16, tag="zdram")          # bf16 z bounce

            nc.sync.dma_start(out=idsb[:], in_=ident[:, :])

            # ---- phase A: build zT = bf16((x / ||x||).T) ------------------
            # Row r = g*1024 + p*8 + t lives in partition p of group-tile g at
            # index t: each partition loads 4KB contiguous per group.
            xfr = xf[:, :].rearrange("(g p t) d -> g p t d", g=8, p=128, t=8)
            zdr = zdram[:, :].rearrange("(g p t) d -> g p t d", g=8, p=128, t=8)
            xgs = []
            for g in range(8):
                xg = W.tile([128, 8, 128], f32, tag="xg", bufs=8)
                xgs.append(xg)
                nc.gpsimd.dma_start(out=xg[:], in_=xfr[g])
                for t in range(8):
                    xsq = W.tile([128, 128], f32, tag="xsq")
                    nc.vector.scalar_tensor_tensor(
                        out=xsq[:],
                        in0=xg[:, t, :],
                        scalar=1.0,
                        in1=xg[:, t, :],
                        op0=OP.mult,
                        op1=OP.mult,
                        accum_out=n2all[:, g * 8 + t : g * 8 + t + 1],
                    )
                if g in (1, 7):
                    # rsqrt in two batches (groups 0-1, then 2-7): the tiny
                    # first batch lets column-block s=0 start earlier while
                    # keeping the ACT table sequence Ln,Exp,Ln,Exp
                    gfirst = 0 if g == 1 else 2
                    b0, bw = gfirst * 8, (g - gfirst + 1) * 8
                    lng = W.tile([128, 48], f32, tag="lng")
                    nc.scalar.activation(
                        out=lng[:, 0:bw], in_=n2all[:, b0 : b0 + bw], func=AF.Ln
                    )
                    nc.scalar.activation(
                        out=scall[:, b0 : b0 + bw],
                        in_=lng[:, 0:bw],
                        func=AF.Exp,
                        scale=-0.5,
                    )
                    for gg in range(gfirst, g + 1):
                        # z_bf16 = x * rsqrt(|x|^2), one broadcast op per group
                        sc = scall[:, gg * 8 : (gg + 1) * 8]
                        scb = bass.AP(
                            tensor=sc.tensor,
                            offset=sc.offset,
                            ap=[list(sc.ap[0]), list(sc.ap[1]), [0, 128]],
                        )
                        zg = W.tile([128, 8, 128], bf16, tag="zg")
                        nc.vector.scalar_tensor_tensor(
                            out=zg[:],
                            in0=xgs[gg][:],
                            scalar=1.0,
                            in1=scb,
                            op0=OP.mult,
                            op1=OP.mult,
                        )
                        nc.gpsimd.dma_start(out=zdr[gg], in_=zg[:])
                        nc.sync.dma_start_transpose(
                            out=zT[:, gg * 1024 : (gg + 1) * 1024],
                            in_=zdram[gg * 1024 : (gg + 1) * 1024, :],
                        )

            # ---- phase B: S row-block, exp, row sums, diagonal positives --
            with tc.tile_pool(name="pmm", bufs=2, space="PSUM") as PM:
                for s in range(NSUP):
                    for m in range(MT):
                        lhsT = zT[:, m * 128 : (m + 1) * 128]
                        ps = PM.tile([128, SUP], f32, tag="ps")
                        for q in range(SUP // 512):
                            nc.tensor.matmul(
                                ps[:, q * 512 : (q + 1) * 512],
                                lhsT,
                                zT[:, s * SUP + q * 512 : s * SUP + (q + 1) * 512],
                                start=True,
                                stop=True,
                            )
                        esc = W.tile([128, SUP], f32, tag="esc")
                        nc.scalar.activation(
                            out=esc[:],
                            in_=ps[:],
                            func=AF.Exp,
                            scale=TEMP_SCALE,
                            accum_out=fcols[:, m * NSUP + s : m * NSUP + s + 1],
                        )
                        # positives: diagonals of the local block-columns of
                        # exp(S) (from SBUF so only ACT touches PSUM banks)
                        for h in range(2):
                            B = 2 * s + h
                            off = h * 1024 + m * 128
                            dsc = W.tile([128, 128], f32, tag="dsc")
                            nc.vector.scalar_tensor_tensor(
                                out=dsc[:],
                                in0=esc[:, off : off + 128],
                                scalar=1.0,
                                in1=idsb[:],
                                op0=OP.mult,
                                op1=OP.mult,
                                accum_out=gbuf[:, m * 8 + B : m * 8 + B + 1],
                            )

                # ---- phase C: batched logsumexp finale --------------------
                # gbuf holds exp(pos logit) per (m-tile, block B); recover the
                # logits with one Ln pass.
                p8 = P.tile([128, MT], f32, tag="p8")
                nc.vector.tensor_reduce(
                    out=p8[:],
                    in_=gbuf[:].rearrange("p (m b) -> p m b", b=8),
                    axis=AX.X,
                    op=OP.add,
                )
                f8 = P.tile([128, MT], f32, tag="f8")
                nc.vector.tensor_reduce(
                    out=f8[:],
                    in_=fcols[:].rearrange("p (m s) -> p m s", s=NSUP),
                    axis=AX.X,
                    op=OP.add,
                )
                a8 = P.tile([128, MT], f32, tag="a8")
                nc.vector.tensor_sub(a8[:], f8[:], p8[:])
                tmp = P.tile([128, 8 * MT], f32, tag="tmp")
                for m in range(MT):
                    nc.vector.tensor_scalar_add(
                        out=tmp[:, m * 8 : (m + 1) * 8],
                        in0=gbuf[:, m * 8 : (m + 1) * 8],
                        scalar1=a8[:, m : m + 1],
                    )
                lnt = P.tile([128, 8 * MT], f32, tag="lnt")
                nc.scalar.activation(out=lnt[:], in_=tmp[:], func=AF.Ln)
                gln = P.tile([128, 8 * MT], f32, tag="gln")
                nc.scalar.activation(out=gln[:], in_=gbuf[:], func=AF.Ln)
                lsc = P.tile([128, 7 * MT], f32, tag="lsc")
                rl = P.tile([128, 1], f32, tag="rl")
                nc.vector.scalar_tensor_tensor(
                    out=lsc[:].rearrange("p (m b) -> p m b", b=7),
                    in0=lnt[:].rearrange("p (m b) -> p m b", b=8)[:, :, 1:8],
                    scalar=1.0,
                    in1=gln[:].rearrange("p (m b) -> p m b", b=8)[:, :, 1:8],
                    op0=OP.mult,
                    op1=OP.subtract,
                    accum_out=rl[:],
                )
                ones = P.tile([128, 1], f32, tag="ones")
                nc.vector.memset(ones[:], 1.0)
                pf = PM.tile([1, 1], f32, tag="ps")
                nc.tensor.matmul(pf[:], ones[:], rl[:], start=True, stop=True)
                osb = P.tile([1, 1], f32, tag="osb")
                nc.vector.tensor_copy(out=osb[:], in_=pf[:])
                nc.sync.dma_start(out=outp[:, :], in_=osb[:])

    nc.compile()
    return nc


def _get_nc():
    if "nc" not in _NC_CACHE:
        _NC_CACHE["nc"] = _build_nc()
    return _NC_CACHE["nc"]


def _make_in_maps(x):
    ident = np.eye(128, dtype=np.float32)
    in_maps = []
    for c in range(NCORES):
        xc = np.ascontiguousarray(np.roll(x, -c * ROWS, axis=0))
        in_maps.append({"xf": xc, "ident": ident})
    return in_maps


def kernel(sim: np.ndarray, _want_results: bool = False, _trace: bool = False):
    x = np.ascontiguousarray(np.asarray(sim, dtype=np.float32).reshape(K, D))
    in_maps = _make_in_maps(x)
    nc = _get_nc()
    from concourse.bass_utils import run_bass_kernel_spmd

    res = run_bass_kernel_spmd(nc, in_maps, list(range(NCORES)), trace=_trace)
    partials = np.array(
        [r["loss_out"][0, 0] for r in res.results], dtype=np.float64
    )
    loss = np.array(partials.sum() / (K * (N - 1)), dtype=np.float32)
    if _want_results:
        return loss, res
    return loss


if __name__ == "__main__":
    nc = _build_nc()
    print("build OK")

